# revision 1
# baseline (speedup 1.0000x reference)
"""Causal multi-head attention (B=4, T=2048, D=1024, H=16) on 8 Trainium2 cores.

Sharding (data + tensor parallel): core c handles batch b = c//2 and head-group
g = c%2 (8 of the 16 heads). Wq/Wk/Wv are column-sharded by head, Wp is
row-sharded; the two per-batch partial outputs are summed on the host (this
replaces the device all-reduce — the host-side sum is the unshard step).

Per-core kernel (all matmuls in float32r = TF32-like, full PE rate at N>=256):
  - everything is computed in "transposed space" to avoid on-chip transposes:
      Qt/Kt [head_dims, t] = W_slice @ x.T       (x passed pre-transposed)
      V     [t, head_dims] = x @ Wv_slice.T      (natural layout)
      St    = scores.T tile blocks [k, q] = Kt_tile.T-contracted with Qt
      E     = exp(St)  (1/sqrt(64) pre-folded into Wq; causal mask applied as
              a 0/1 multiply on the diagonal 128x128 zones; k-tiles fully
              above the diagonal are skipped, diagonal tiles are narrowed)
      ctxT_aug [65, q] = Vaug.T @ E  accumulated over k-tiles in PSUM, where
              Vaug carries a ones-column so row 64 is the softmax denominator
      ctx   = ctxT_aug[0:64] * broadcast(1/denominator)   (PE ones-broadcast)
      outT  = WpT_slice.T @ ctx  -> partial output, summed on host
  - St tiles are computed into PSUM bank-pairs [128, 1024] so one ACT exp
    covers two k-tiles (halves the ACT per-op overhead); the two heads of a
    head-pair are interleaved to hide exp latency behind PE work.
"""
import numpy as np

T = 2048
D = 1024
B = 4
H = 16
HL = 8            # heads per core
NP = 4            # head pairs per core
QB = 512          # q-block width (one PSUM bank of fp32)
NQB = T // QB
NKT = T // 128

_COMPILED = None


# --------------------------------------------------------------------------
# bass kernel build
# --------------------------------------------------------------------------
def _build_bass():
    import concourse.bass as bass
    import concourse.mybir as mybir
    from concourse.tile import TileContext

    F32 = mybir.dt.float32
    F32R = mybir.dt.float32r
    Act = mybir.ActivationFunctionType
    Alu = mybir.AluOpType

    nc = bass.Bass()
    xt = nc.dram_tensor("xt", [D, T], F32R, kind="ExternalInput")
    wq = nc.dram_tensor("wq", [D, 512], F32R, kind="ExternalInput")
    wk = nc.dram_tensor("wk", [D, 512], F32R, kind="ExternalInput")
    wv = nc.dram_tensor("wv", [D, 512], F32R, kind="ExternalInput")
    wp = nc.dram_tensor("wp", [512, D], F32R, kind="ExternalInput")
    mask1 = nc.dram_tensor("mask1", [128, 128], F32R, kind="ExternalInput")
    ones64 = nc.dram_tensor("ones64", [1, 64], F32R, kind="ExternalInput")
    vones = nc.dram_tensor("vones", [128, NKT * HL], F32R, kind="ExternalInput")
    outt = nc.dram_tensor("outt", [D, T], F32, kind="ExternalOutput")

    with TileContext(nc) as tc, nc.allow_low_precision(reason="f32r pipeline"):
        with tc.tile_pool(name="wts", bufs=1) as wts, \
             tc.tile_pool(name="xp", bufs=2) as xp, \
             tc.tile_pool(name="big", bufs=1) as big, \
             tc.tile_pool(name="qtp", bufs=5) as qtp, \
             tc.tile_pool(name="ctp", bufs=4) as ctp, \
             tc.tile_pool(name="ep", bufs=4) as ep, \
             tc.tile_pool(name="sm", bufs=1) as sm, \
             tc.tile_pool(name="osb", bufs=2) as osb, \
             tc.tile_pool(name="pmm", bufs=3, space="PSUM") as pmm, \
             tc.tile_pool(name="pca", bufs=2, space="PSUM") as pca:

            # weights/constants; DMAs split per k-slice so compute starts as
            # soon as the first slices land
            wq_t = wts.tile([128, 8, 512], F32R, tag="wq")
            wk_t = wts.tile([128, 8, 512], F32R, tag="wk")
            wv_t = wts.tile([128, 8, 512], F32R, tag="wv")
            wp_t = wts.tile([128, 4, 1024], F32R, tag="wp")
            wqr = wq[:].rearrange("(n p) m -> p n m", p=128)
            wkr = wk[:].rearrange("(n p) m -> p n m", p=128)
            wvr = wv[:].rearrange("(n p) m -> p n m", p=128)
            x_tiles = {}

            def load_x(tb):
                x_t = xp.tile([128, 8, QB], F32R, tag="x", name=f"x_t{tb}a")
                xr = xt[:, QB * tb:QB * (tb + 1)].rearrange("(n p) m -> p n m", p=128)
                for kk in range(8):
                    nc.sync.dma_start(x_t[:, kk, :], xr[:, kk, :])
                x_tiles[tb] = x_t

            xr0 = xt[:, 0:QB].rearrange("(n p) m -> p n m", p=128)
            x_t0 = xp.tile([128, 8, QB], F32R, tag="x", name="x_t0")
            for kk in range(8):
                nc.sync.dma_start(wq_t[:, kk, :], wqr[:, kk, :])
                nc.sync.dma_start(x_t0[:, kk, :], xr0[:, kk, :])
            x_tiles[0] = x_t0
            for kk in range(8):
                nc.sync.dma_start(wk_t[:, kk, :], wkr[:, kk, :])
            for kk in range(8):
                nc.sync.dma_start(wv_t[:, kk, :], wvr[:, kk, :])
            m1 = sm.tile([128, 128], F32R, tag="m1")
            nc.sync.dma_start(m1[:], mask1[:])
            on = sm.tile([1, 64], F32R, tag="on")
            nc.sync.dma_start(on[:], ones64[:])

            kt_t = big.tile([128, NP, T], F32R, tag="kt")
            va_t = big.tile([128, NKT, HL, 65], F32R, tag="va")
            nc.sync.dma_start(
                va_t[:, :, :, 64:65].squeeze(3),
                vones[:].rearrange("p (n h) -> p n h", n=NKT))
            wpr = wp[:].rearrange("(n p) m -> p n m", p=128)
            for kk in range(4):
                nc.sync.dma_start(wp_t[:, kk, :], wpr[:, kk, :])

            ctx_tiles = {}

            for tb in range(NQB):
                x_t = x_tiles[tb]
                if tb + 1 < NQB:
                    load_x(tb + 1)

                # projections for this t-block
                qt_tiles = {}
                for p in range(NP):
                    ps_q = pmm.tile([128, 2 * QB], F32, tag="mm", name=f"psq{tb}_{p}")
                    for kk in range(8):
                        nc.tensor.matmul(ps_q[:, 0:QB], wq_t[:, kk, 128 * p:128 * (p + 1)],
                                         x_t[:, kk, :], start=(kk == 0), stop=(kk == 7))
                    q_tile = qtp.tile([128, QB], F32R, tag="qt", name=f"qt{tb}_{p}")
                    nc.vector.tensor_copy(q_tile[:], ps_q[:, 0:QB])
                    qt_tiles[p] = q_tile
                for p in range(NP):
                    ps_k = pmm.tile([128, 2 * QB], F32, tag="mm", name=f"psk{tb}_{p}")
                    for kk in range(8):
                        nc.tensor.matmul(ps_k[:, 0:QB], wk_t[:, kk, 128 * p:128 * (p + 1)],
                                         x_t[:, kk, :], start=(kk == 0), stop=(kk == 7))
                    nc.vector.tensor_copy(kt_t[:, p, QB * tb:QB * (tb + 1)], ps_k[:, 0:QB])
                for tt in range(4):
                    ps_v = pmm.tile([128, 2 * QB], F32, tag="mm", name=f"psv{tb}_{tt}")
                    for kk in range(8):
                        nc.tensor.matmul(ps_v[:, 0:QB], x_t[:, kk, 128 * tt:128 * (tt + 1)],
                                         wv_t[:, kk, :], start=(kk == 0), stop=(kk == 7))
                    nc.vector.tensor_copy(
                        va_t[:, 4 * tb + tt, :, 0:64],
                        ps_v[:, 0:QB].rearrange("p (h d) -> p h d", h=HL))

                # attention for q-block j = tb
                j = tb
                nkt_j = 4 * j + 4
                for p in range(NP):
                    ctx_tile = ctp.tile([128, QB], F32R, tag="ctx", name=f"ctx{j}_{p}")
                    ctx_tiles[(p, j)] = ctx_tile
                    q_tile = qt_tiles[p]
                    ctxa = [pca.tile([65, QB], F32, tag="ctxa", name=f"ctxa_{j}_{p}_{s2}")
                            for s2 in range(2)]
                    for ip in range(nkt_j // 2):
                        i0, i1 = 2 * ip, 2 * ip + 1
                        o0, o1 = i0 - 4 * j, i1 - 4 * j
                        cs0 = 0 if o0 < 0 else min(128 * o0, 256)
                        cs1 = 0 if o1 < 0 else min(128 * o1, 256)
                        for s in range(2):
                            h = 2 * p + s
                            hs = slice(64 * s, 64 * s + 64)
                            tp = (64 * s, 0)
                            st2 = pmm.tile([128, 2 * QB], F32, tag="mm",
                                           name=f"st{j}_{p}_{ip}_{s}")
                            nc.tensor.matmul(st2[:, cs0:QB],
                                             kt_t[hs, p, 128 * i0:128 * (i0 + 1)],
                                             q_tile[hs, cs0:QB],
                                             start=True, stop=True, tile_position=tp)
                            nc.tensor.matmul(st2[:, QB + cs1:2 * QB],
                                             kt_t[hs, p, 128 * i1:128 * (i1 + 1)],
                                             q_tile[hs, cs1:QB],
                                             start=True, stop=True, tile_position=tp)
                            e2 = ep.tile([128, 2 * QB], F32R, tag="e",
                                         name=f"e{j}_{p}_{ip}_{s}")
                            if o1 < 0:
                                nc.scalar.activation(e2[:], st2[:], Act.Exp)
                            else:
                                nc.scalar.activation(e2[:, cs0:QB], st2[:, cs0:QB], Act.Exp)
                                nc.scalar.activation(e2[:, QB + cs1:2 * QB],
                                                     st2[:, QB + cs1:2 * QB], Act.Exp)
                                for (oo, base) in ((o0, 0), (o1, QB)):
                                    if oo < 0:
                                        continue
                                    if oo < 3:
                                        z = slice(base + 128 * oo, base + 128 * (oo + 1))
                                        nc.vector.tensor_tensor(e2[:, z], e2[:, z], m1[:],
                                                                op=Alu.mult)
                                    else:
                                        zz = slice(base + 256, base + 384)
                                        nc.vector.tensor_scalar_mul(e2[:, zz], e2[:, zz], 0.0)
                                        z = slice(base + 384, base + QB)
                                        nc.vector.tensor_tensor(e2[:, z], e2[:, z], m1[:],
                                                                op=Alu.mult)
                            nc.tensor.matmul(ctxa[s][:, cs0:QB], va_t[:, i0, h, :],
                                             e2[:, cs0:QB], start=(i0 == 0), stop=False)
                            nc.tensor.matmul(ctxa[s][:, cs1:QB], va_t[:, i1, h, :],
                                             e2[:, QB + cs1:2 * QB],
                                             start=False, stop=(i1 == nkt_j - 1))
                    for s in range(2):
                        # copy raw ctx + reciprocal out of PSUM (frees the
                        # ctxa bank), then PE-broadcast the reciprocal across
                        # partitions and normalize
                        recip = sm.tile([1, QB], F32R, tag="recip", bufs=2,
                                        name=f"recip{j}_{p}_{s}")
                        nc.vector.reciprocal(recip[:], ctxa[s][64:65, :])
                        raw = sm.tile([64, QB], F32, tag="raw", bufs=2,
                                      name=f"raw{j}_{p}_{s}")
                        nc.vector.tensor_copy(raw[:], ctxa[s][0:64, :])
                        bcp = pca.tile([128, QB], F32, tag="ctxa", name=f"bcp{j}_{p}_{s}")
                        nc.tensor.matmul(bcp[0:64, :], on[:], recip[:], start=True, stop=True)
                        nc.vector.tensor_tensor(ctx_tile[64 * s:64 * s + 64, :],
                                                raw[:], bcp[0:64, :], op=Alu.mult)

                # output projection for this q-block
                for m in range(8):
                    pf = pca.tile([128, QB], F32, tag="ctxa", name=f"pf{j}_{m}")
                    for p in range(NP):
                        nc.tensor.matmul(pf[:, 0:QB], wp_t[:, p, 128 * m:128 * (m + 1)],
                                         ctx_tiles[(p, j)][:], start=(p == 0), stop=(p == 3))
                    ob = osb.tile([128, QB], F32, tag="ob", name=f"ob{j}_{m}")
                    nc.vector.tensor_copy(ob[:], pf[:, 0:QB])
                    nc.sync.dma_start(outt[128 * m:128 * (m + 1), QB * j:QB * (j + 1)], ob[:])
    return nc


def _split_waits(nc, limit=1):
    """This walrus build accepts only one sync wait per TPB_CTRL instruction;
    move excess waits onto preceding same-engine NOPs."""
    import concourse.mybir as mybir
    for f in nc.m.functions:
        for bb in f.blocks:
            new_insts = []
            for inst in bb.instructions:
                si = inst.sync_info
                if si is not None and si.on_wait and len(si.on_wait) > limit:
                    waits = list(si.on_wait)
                    k = 0
                    while len(waits) - k > limit:
                        chunk = waits[k:k + limit]
                        k += limit
                        nop = mybir.InstNoOp(name=f"{inst.name}_ws{k}")
                        nop.engine = inst.engine
                        nop.sync_info = mybir.SyncInfo(on_wait=chunk, on_update=[])
                        new_insts.append(nop)
                    si.on_wait = waits[k:]
                new_insts.append(inst)
            bb.instructions = new_insts


# --------------------------------------------------------------------------
# compile + SPMD execution via PJRT (axon) — jit once, reuse
# --------------------------------------------------------------------------
class _Compiled:
    def __init__(self, n_cores=8):
        import jax
        from jax.sharding import Mesh, PartitionSpec
        from jax.experimental.shard_map import shard_map
        import concourse.mybir as mybir
        from concourse.bass2jax import (_bass_exec_p, install_neuronx_cc_hook,
                                        partition_id_tensor)

        nc = _build_bass()
        _split_waits(nc)
        install_neuronx_cc_hook()
        partition_name = nc.partition_id_tensor.name if nc.partition_id_tensor else None
        in_names, out_names, out_avals, zero_outs = [], [], [], []
        for alloc in nc.m.functions[0].allocations:
            if not isinstance(alloc, mybir.MemoryLocationSet):
                continue
            name = alloc.memorylocations[0].name
            if alloc.kind == "ExternalInput":
                if name != partition_name:
                    in_names.append(name)
            elif alloc.kind == "ExternalOutput":
                shape = tuple(alloc.tensor_shape)
                dtype = mybir.dt.np(alloc.dtype)
                out_names.append(name)
                out_avals.append(jax.core.ShapedArray(shape, dtype))
                zero_outs.append(np.zeros(shape, dtype))
        n_params = len(in_names)
        all_in_names = list(in_names) + list(out_names)
        if partition_name is not None:
            all_in_names.append(partition_name)

        def _body(*args):
            operands = list(args)
            if partition_name is not None:
                operands.append(partition_id_tensor())
            outs = _bass_exec_p.bind(
                *operands,
                out_avals=tuple(out_avals),
                in_names=tuple(all_in_names),
                out_names=tuple(out_names),
                lowering_input_output_aliases=(),
                sim_require_finite=True,
                sim_require_nnan=True,
                nc=nc,
            )
            return tuple(outs)

        devices = jax.devices()[:n_cores]
        assert len(devices) >= n_cores, f"need {n_cores} cores, have {len(devices)}"
        self.n_cores = n_cores
        self.in_names, self.out_names = in_names, out_names
        self.out_avals, self.zero_outs = out_avals, zero_outs
        mesh = Mesh(np.asarray(devices[:n_cores]), ("core",))
        in_specs = (PartitionSpec("core"),) * (n_params + len(out_names))
        out_specs = (PartitionSpec("core"),) * len(out_names)
        self.fn = jax.jit(
            shard_map(_body, mesh=mesh, in_specs=in_specs,
                      out_specs=out_specs, check_rep=False),
            keep_unused=True)

    def run(self, in_maps):
        import jax
        args = []
        for name in self.in_names:
            args.append(np.concatenate([np.asarray(m[name]) for m in in_maps], axis=0))
        for z in self.zero_outs:
            args.append(np.zeros((self.n_cores * z.shape[0], *z.shape[1:]), z.dtype))
        outs = self.fn(*args)
        jax.block_until_ready(outs)
        res = []
        for c in range(self.n_cores):
            d = {}
            for i, name in enumerate(self.out_names):
                a = np.asarray(outs[i]).reshape(self.n_cores, *self.out_avals[i].shape)[c]
                d[name] = a
            res.append(d)
        return res


# --------------------------------------------------------------------------
# host-side shard / unshard
# --------------------------------------------------------------------------
def _make_core_inputs(x, Wq, Wk, Wv, Wp, core):
    g = core % 2
    b = core // 2
    rows = slice(512 * g, 512 * (g + 1))
    kl = np.arange(128)
    return {
        "xt": np.ascontiguousarray(x[b].T.astype(np.float32)),
        # fold the 1/sqrt(head_dim) score scale into Wq
        "wq": np.ascontiguousarray((Wq[rows, :] * 0.125).T.astype(np.float32)),
        "wk": np.ascontiguousarray(Wk[rows, :].T.astype(np.float32)),
        "wv": np.ascontiguousarray(Wv[rows, :].T.astype(np.float32)),
        "wp": np.ascontiguousarray(Wp[:, rows].T.astype(np.float32)),
        "mask1": (kl[:, None] <= kl[None, :]).astype(np.float32),
        "ones64": np.ones((1, 64), np.float32),
        "vones": np.ones((128, NKT * HL), np.float32),
    }


def kernel(x, Wq, Wk, Wv, Wp):
    """Full-input / full-output causal MHA. x: (4, 2048, 1024) fp32;
    Wq/Wk/Wv/Wp: (1024, 1024) fp32. Returns (4, 2048, 1024) fp32."""
    global _COMPILED
    x = np.asarray(x, dtype=np.float32)
    Wq = np.asarray(Wq, dtype=np.float32)
    Wk = np.asarray(Wk, dtype=np.float32)
    Wv = np.asarray(Wv, dtype=np.float32)
    Wp = np.asarray(Wp, dtype=np.float32)
    assert x.shape == (B, T, D), x.shape

    if _COMPILED is None:
        _COMPILED = _Compiled(8)
    in_maps = [_make_core_inputs(x, Wq, Wk, Wv, Wp, c) for c in range(8)]
    results = _COMPILED.run(in_maps)

    out = np.empty((B, T, D), np.float32)
    for b in range(B):
        acc = results[2 * b]["outt"] + results[2 * b + 1]["outt"]
        out[b] = acc.T
    return out
